# revision 2
# baseline (speedup 1.0000x reference)
"""Differentiable-histogram Trainium2 kernel (256 triangular bins).

hist[b, j] = sum_i relu(1 - |255*x_bi - j|), one image per core.

v4 design (vs the v3 baseline):
  tb  = 255*x - 8               ACT (f32 -> fp16)
  h16 = round16(tb)              DVE TS (+M, -M) magic (f32 ALU), 4x mode
  u   = tb - h16 in [-8, 8]      DVE TT subtract, 2x  (u = r - 8)
  a   = (h16 == 16*h')           DVE TT is_equal, one-hot (16 slots)
  d   = r - l = u - (l-8)        split: NSLOT_ACT slots on ScalarE via
                                 Copy(bias=8-l), the rest one DVE TT vs a
                                 shifted iota (anchor slot iota 4096)
  cl  = clamp(d, -1, 1)          DVE TS (min 1, max -1), 4x
  G*  = A^T cl                   single PE matmul per group. Host recovery:
                                 cl = 1 - 2*cumsum(B) + B per pixel, so with
                                 N_h = -G*[h,17], an alternating prefix
                                 recursion yields G = A^T B exactly.
  Pair-interleaved (P, G/2, W, 2) one-hot layout as in v3 (DVE 2x_1p mode,
  block+parity-diagonal PSUM extraction on host). FOLD=8 pixels per matmul
  (128-col lhsT, 136-col rhs) halves PE instruction count.
"""

import json as _json
from contextlib import ExitStack

import numpy as np

import concourse.bass as bass
import concourse.tile as tile
from concourse import mybir
from concourse.bass_utils import run_bass_kernel_spmd

FP32 = mybir.dt.float32
FP16 = mybir.dt.float16
ALU = mybir.AluOpType
ACT = mybir.ActivationFunctionType

N_CORES = 8
P, F = 128, 1536  # per-core pixels: 3*256*256 = 196608 = 128*1536
N_PSUM = 4
FOLD = 8  # pixels folded per matmul: lhsT 16*FOLD/2... fold2 = FOLD//2 pair-groups
BUFS = 3
# (chunk_size, n_act_slots): small ramp chunks keep d fully on DVE (the
# per-instruction ScalarE overhead dominates at small G)
NSLOT_ACT = 13
CHUNKS = [(32, 0), (64, 0), (512, NSLOT_ACT), (512, NSLOT_ACT), (352, 10), (64, 0)]
MAGIC16 = 201326592.0  # 1.5 * 2**27 -- f32-ALU round-to-multiple-of-16
NFINE = 18  # 17 triangle slots + one constant -1 anchor column


def _split_multiwaits(bir_bytes: bytes) -> bytes:
    """This container's walrus rejects any instruction carrying more than
    one sem wait. Split extras onto standalone EventSemaphore instructions;
    additionally drop the exit-drain's queue waits (NRT drains rings at
    exec end anyway)."""
    bir = _json.loads(bir_bytes)
    for fn in bir["functions"]:
        for blk in fn["blocks"]:
            is_end = str(blk.get("name", "")).endswith("_end")
            out = []
            for ins in blk["instructions"]:
                si = ins.get("sync_info")
                ow = (si or {}).get("on_wait") or []
                if is_end and ins.get("opcode") == "Drain" and len(ow) > 1:
                    si["on_wait"] = []
                elif len(ow) > 1:
                    for k, w in enumerate(ow[:-1]):
                        out.append(
                            {
                                "debug": ins.get("debug", 1),
                                "engine": ins["engine"],
                                "ins": [],
                                "name": f"{ins['name']}_w{k}",
                                "opcode": "EventSemaphore",
                                "outs": [],
                                "sync_info": {"on_update": [], "on_wait": [w]},
                            }
                        )
                    si["on_wait"] = [ow[-1]]
                out.append(ins)
            blk["instructions"] = out
    return _json.dumps(bir).encode()


def _build_program():
    assert sum(g for g, _ in CHUNKS) == F
    fold2 = FOLD // 2
    for g, _ in CHUNKS:
        assert g % (2 * fold2) == 0
    MR, MC = 16 * fold2 * 2, NFINE * fold2 * 2
    n_mm = (F // 2) // fold2

    nc = bass.Bass("TRN2", target_bir_lowering=False)

    x_dram = nc.dram_tensor("x", [P, F], FP32, kind="ExternalInput")
    gacc_dram = nc.dram_tensor("gacc", [N_PSUM, MR, MC], FP32, kind="ExternalOutput")

    iota_a_np = np.repeat(16.0 * np.arange(16, dtype=np.float32), 2).reshape(16, 2)
    iota_b_np = np.concatenate([np.arange(17, dtype=np.float32) - 8.0, [4096.0]])
    iota_b_np = np.repeat(iota_b_np, 2).reshape(NFINE, 2)
    iota_a_np = np.broadcast_to(iota_a_np[None], (P, 16, 2)).astype(np.float16)
    iota_b_np = np.broadcast_to(iota_b_np[None], (P, NFINE, 2)).astype(np.float16)
    iota_a_dram = nc.inline_tensor(np.ascontiguousarray(iota_a_np), "iota_a")
    iota_b_dram = nc.inline_tensor(np.ascontiguousarray(iota_b_np), "iota_b")

    with tile.TileContext(nc) as tc, ExitStack() as ctx:
        singles = ctx.enter_context(tc.tile_pool(name="singles", bufs=1))
        pool = ctx.enter_context(tc.tile_pool(name="work", bufs=BUFS))
        spool = ctx.enter_context(tc.tile_pool(name="small", bufs=6))
        bpool = ctx.enter_context(tc.tile_pool(name="bwork", bufs=2))
        psum_pool = ctx.enter_context(tc.tile_pool(name="psum", bufs=1, space="PSUM"))

        iota_a = singles.tile([P, 16, 2], FP16)
        iota_b = singles.tile([P, NFINE, 2], FP16)
        iota_loaded = [False]

        psums = []
        for i in range(N_PSUM):
            ps = psum_pool.tile([MR, MC], FP32, tag=f"ps{i}", name=f"ps{i}")
            psums.append(ps)

        def src_pairs(ap, w, g2):
            # (P, Gc) tile viewed as (P, g2, w, 2): pairs inner, bcast w
            return bass.AP(
                tensor=ap.tensor,
                offset=ap.offset,
                ap=[ap.ap[0], [2, g2], [0, w], [1, 2]],
            )

        def iota_bcast(ap, w, g2, w0=0):
            # (P, w, 2) iota tile viewed as (P, g2, w, 2), slots [w0, w0+w)
            return bass.AP(
                tensor=ap.tensor,
                offset=ap.offset + 2 * w0,
                ap=[ap.ap[0], [0, g2], [2, w], [1, 2]],
            )

        x_off = 0
        mi = 0
        pending = None  # (a_t, d_t, G2) of the previous chunk
        def flush(pend):
            # cl + matmuls for a finished chunk; emitted one chunk late so
            # the in-order DVE never stalls waiting on ScalarE's d-slices.
            nonlocal mi
            a_t, d_t, G2, p2 = pend
            b_t = bpool.tile([P, G2, NFINE, 2], FP16, tag="b_t")
            if p2:
                nc.gpsimd.tensor_scalar(
                    b_t[:, :p2], d_t[:, :p2], 1.0, -1.0, ALU.min, ALU.max
                )
            nc.vector.tensor_scalar(
                b_t[:, p2:], d_t[:, p2:], 1.0, -1.0, ALU.min, ALU.max
            )
            for gb in range(0, G2, fold2):
                nc.tensor.matmul(
                    psums[mi % N_PSUM][:],
                    a_t[:, gb : gb + fold2, :, :],
                    b_t[:, gb : gb + fold2, :, :],
                    start=(mi < N_PSUM),
                    stop=(mi >= n_mm - N_PSUM),
                )
                mi += 1

        for c, (Gc, k_act) in enumerate(CHUNKS):
            G2 = Gc // 2
            p2 = 0  # GpSimd clamp disabled: its SBUF traffic stalls DVE
            xc = pool.tile([P, Gc], FP32, tag="xc")
            nc.sync.dma_start(xc[:], x_dram[:, x_off : x_off + Gc])
            x_off += Gc
            if not iota_loaded[0]:
                # after the first pixel DMA so chunk 0 hits ScalarE sooner
                nc.sync.dma_start(iota_a[:], iota_a_dram[:])
                nc.sync.dma_start(iota_b[:], iota_b_dram[:])
                iota_loaded[0] = True

            tb = spool.tile([P, Gc], FP16, tag="tb")
            if k_act:
                nc.scalar.activation(tb[:], xc[:], ACT.Copy, scale=255.0, bias=-8.0)
            else:
                nc.vector.tensor_scalar(
                    tb[:], xc[:], 255.0, 8.0, ALU.mult, ALU.subtract
                )
            h16 = spool.tile([P, Gc], FP16, tag="h16")
            nc.vector.tensor_scalar(
                h16[:], tb[:], MAGIC16, MAGIC16, ALU.add, ALU.subtract
            )
            u = spool.tile([P, Gc], FP16, tag="u")
            nc.vector.tensor_tensor(u[:], tb[:], h16[:], ALU.subtract)

            # d[l] = r - l = u - (l - 8): ScalarE owns the top k_act slots
            # (incl. the anchor) as Copy(bias=8-l); DVE owns the rest.
            ndve = NFINE - k_act
            d_t = pool.tile([P, G2, NFINE, 2], FP16, tag="d_t")
            for l in range(ndve, NFINE):
                bias = -4088.0 if l == NFINE - 1 else float(8 - l)
                nc.scalar.activation(
                    d_t[:, :, l, :], src_pairs(u[:], 1, G2), ACT.Copy, bias=bias
                )

            a_t = pool.tile([P, G2, 16, 2], FP16, tag="a_t")
            nc.vector.tensor_tensor(
                a_t[:],
                src_pairs(h16[:], 16, G2),
                iota_bcast(iota_a[:], 16, G2),
                ALU.is_equal,
            )
            nc.vector.tensor_tensor(
                d_t[:, :, :ndve, :],
                src_pairs(u[:], ndve, G2),
                iota_bcast(iota_b[:], ndve, G2),
                ALU.subtract,
            )

            if pending is not None:
                flush(pending)
            pending = (a_t, d_t, G2, p2)
        flush(pending)
        assert mi == n_mm, (mi, n_mm)

        # PSUM -> SBUF stage (DVE/ACT alternate) -> one DMA out.
        stage = singles.tile([MR, N_PSUM, MC], FP32)
        for i in range(N_PSUM):
            nc.scalar.activation(stage[:, i, :], psums[i][:], ACT.Copy)
        nc.sync.dma_start(gacc_dram.rearrange("n r c -> r n c"), stage[:])

    orig = nc.to_json_bytes
    nc.to_json_bytes = lambda *a, **k: _split_multiwaits(orig(*a, **k))
    return nc


def _gacc_to_hist(gacc: np.ndarray) -> np.ndarray:
    """(N_PSUM, 16*FOLD, 18*FOLD) raw accumulators -> (256,) histogram.

    Device accumulated G*[h,l] = sum_i A[i,h] * clamp(r_i - l, -1, 1) for
    l = 0..16, and G*[h,17] = -N_h (anchor column). Per pixel
    cl = 1 - 2*cumsum(B) + B, so PS[l] = (N_h - G*[l]) - PS[l-1] gives the
    prefix sums of G = A^T B, recovered by differencing."""
    fold2 = FOLD // 2
    acc = gacc.astype(np.float64).sum(axis=0)
    gst = np.zeros((16, NFINE), np.float64)
    for pb in range(fold2):
        for par in range(2):
            gst += acc[
                pb * 32 + par : pb * 32 + 32 : 2,
                pb * 2 * NFINE + par : pb * 2 * NFINE + 2 * NFINE : 2,
            ]
    nh = -gst[:, 17]
    ps = np.zeros((16, 17), np.float64)
    prev = np.zeros(16)
    for l in range(17):
        ps[:, l] = (nh - gst[:, l]) - prev
        prev = ps[:, l]
    g = np.diff(np.concatenate([np.zeros((16, 1)), ps], axis=1), axis=1)
    hist = g[:, :16].copy()
    hist[1:, 0] += g[:-1, 16]
    return hist.reshape(256).astype(np.float32)


_NC_CACHE = []


def kernel(images_batch: np.ndarray, bin_centers: np.ndarray) -> np.ndarray:
    images = np.asarray(images_batch, dtype=np.float32)
    assert images.shape == (N_CORES, 3, 256, 256), images.shape
    # bin_centers is linspace(0,1,256) by construction; the kernel math
    # hardcodes those bins (t = 255*x vs integer bin index).

    if not _NC_CACHE:
        _NC_CACHE.append(_build_program())
    nc = _NC_CACHE[0]

    in_maps = [{"x": images[b].reshape(P, F).copy()} for b in range(N_CORES)]
    res = run_bass_kernel_spmd(nc, in_maps, core_ids=list(range(N_CORES)))
    return np.stack([_gacc_to_hist(res.results[b]["gacc"]) for b in range(N_CORES)])


if __name__ == "__main__":
    rng = np.random.default_rng(1)
    imgs = rng.random((8, 3, 256, 256), dtype=np.float32)
    bins = np.linspace(0.0, 1.0, 256, dtype=np.float32)
    out = kernel(images_batch=imgs, bin_centers=bins)
    t = imgs.reshape(8, -1).astype(np.float64) * 255.0
    j = np.arange(256)
    want = np.clip(1.0 - np.abs(t[:, :, None] - j[None, None, :]), 0, None).sum(1)
    rel = np.abs(out - want).max() / np.abs(want).max()
    print("self-test rel err:", rel)
    print("PASS" if rel < 2e-2 else "FAIL")


# revision 3
# speedup vs baseline: 1.0443x; 1.0443x over previous
"""Differentiable-histogram Trainium2 kernel (256 triangular bins).

hist[b, j] = sum_i relu(1 - |255*x_bi - j|), one image per core.

v4 design (vs the v3 baseline):
  tb  = 255*x - 8               ACT (f32 -> fp16)
  h16 = round16(tb)              DVE TS (+M, -M) magic (f32 ALU), 4x mode
  u   = tb - h16 in [-8, 8]      DVE TT subtract, 2x  (u = r - 8)
  a   = (h16 == 16*h')           DVE TT is_equal, one-hot (16 slots)
  d   = r - l = u - (l-8)        split: NSLOT_ACT slots on ScalarE via
                                 Copy(bias=8-l), the rest one DVE TT vs a
                                 shifted iota (anchor slot iota 4096)
  cl  = clamp(d, -1, 1)          DVE TS (min 1, max -1), 4x
  G*  = A^T cl                   single PE matmul per group. Host recovery:
                                 cl = 1 - 2*cumsum(B) + B per pixel, so with
                                 N_h = -G*[h,17], an alternating prefix
                                 recursion yields G = A^T B exactly.
  Pair-interleaved (P, G/2, W, 2) one-hot layout as in v3 (DVE 2x_1p mode,
  block+parity-diagonal PSUM extraction on host). FOLD=8 pixels per matmul
  (128-col lhsT, 136-col rhs) halves PE instruction count.
"""

import json as _json
from contextlib import ExitStack

import numpy as np

import concourse.bass as bass
import concourse.tile as tile
from concourse import mybir
from concourse.bass_utils import run_bass_kernel_spmd

FP32 = mybir.dt.float32
FP16 = mybir.dt.float16
ALU = mybir.AluOpType
ACT = mybir.ActivationFunctionType

N_CORES = 8
P, F = 128, 1536  # per-core pixels: 3*256*256 = 196608 = 128*1536
N_PSUM = 4
FOLD = 8  # pixels folded per matmul: lhsT 16*FOLD/2... fold2 = FOLD//2 pair-groups
BUFS = 3
# (chunk_size, n_act_slots): small ramp chunks keep d fully on DVE (the
# per-instruction ScalarE overhead dominates at small G)
NSLOT_ACT = 13
CHUNKS = [(32, 0), (64, 0), (512, NSLOT_ACT), (512, NSLOT_ACT), (352, 10), (64, 0)]
MAGIC16 = 201326592.0  # 1.5 * 2**27 -- f32-ALU round-to-multiple-of-16
NFINE = 18  # 17 triangle slots + one constant -1 anchor column


def _split_multiwaits(bir_bytes: bytes) -> bytes:
    """This container's walrus rejects any instruction carrying more than
    one sem wait. Split extras onto standalone EventSemaphore instructions;
    additionally drop the exit-drain's queue waits (NRT drains rings at
    exec end anyway)."""
    bir = _json.loads(bir_bytes)
    for fn in bir["functions"]:
        for blk in fn["blocks"]:
            is_end = str(blk.get("name", "")).endswith("_end")
            out = []
            for ins in blk["instructions"]:
                si = ins.get("sync_info")
                ow = (si or {}).get("on_wait") or []
                if is_end and ins.get("opcode") == "Drain" and len(ow) > 1:
                    si["on_wait"] = []
                elif len(ow) > 1:
                    for k, w in enumerate(ow[:-1]):
                        out.append(
                            {
                                "debug": ins.get("debug", 1),
                                "engine": ins["engine"],
                                "ins": [],
                                "name": f"{ins['name']}_w{k}",
                                "opcode": "EventSemaphore",
                                "outs": [],
                                "sync_info": {"on_update": [], "on_wait": [w]},
                            }
                        )
                    si["on_wait"] = [ow[-1]]
                out.append(ins)
            blk["instructions"] = out
    return _json.dumps(bir).encode()


def _build_program():
    assert sum(g for g, _ in CHUNKS) == F
    fold2 = FOLD // 2
    for g, _ in CHUNKS:
        assert g % (2 * fold2) == 0
    MR, MC = 16 * fold2 * 2, NFINE * fold2 * 2
    n_mm = (F // 2) // fold2

    nc = bass.Bass("TRN2", target_bir_lowering=False)

    x_dram = nc.dram_tensor("x", [P, F], FP32, kind="ExternalInput")
    gacc_dram = nc.dram_tensor("gacc", [N_PSUM, MR, MC], FP32, kind="ExternalOutput")

    iota_a_np = np.repeat(16.0 * np.arange(16, dtype=np.float32), 2).reshape(16, 2)
    iota_b_np = np.concatenate([np.arange(17, dtype=np.float32) - 8.0, [4096.0]])
    iota_b_np = np.repeat(iota_b_np, 2).reshape(NFINE, 2)
    iota_a_np = np.broadcast_to(iota_a_np[None], (P, 16, 2)).astype(np.float16)
    iota_b_np = np.broadcast_to(iota_b_np[None], (P, NFINE, 2)).astype(np.float16)
    iota_a_dram = nc.inline_tensor(np.ascontiguousarray(iota_a_np), "iota_a")
    iota_b_dram = nc.inline_tensor(np.ascontiguousarray(iota_b_np), "iota_b")

    with tile.TileContext(nc) as tc, ExitStack() as ctx:
        singles = ctx.enter_context(tc.tile_pool(name="singles", bufs=1))
        pool = ctx.enter_context(tc.tile_pool(name="work", bufs=BUFS))
        spool = ctx.enter_context(tc.tile_pool(name="small", bufs=6))
        bpool = ctx.enter_context(tc.tile_pool(name="bwork", bufs=2))
        psum_pool = ctx.enter_context(tc.tile_pool(name="psum", bufs=1, space="PSUM"))

        iota_a = singles.tile([P, 16, 2], FP16)
        iota_b = singles.tile([P, NFINE, 2], FP16)
        iota_loaded = [False]

        psums = []
        for i in range(N_PSUM):
            ps = psum_pool.tile([MR, MC], FP32, tag=f"ps{i}", name=f"ps{i}")
            psums.append(ps)

        def src_pairs(ap, w, g2):
            # (P, Gc) tile viewed as (P, g2, w, 2): pairs inner, bcast w
            return bass.AP(
                tensor=ap.tensor,
                offset=ap.offset,
                ap=[ap.ap[0], [2, g2], [0, w], [1, 2]],
            )

        def iota_bcast(ap, w, g2, w0=0):
            # (P, w, 2) iota tile viewed as (P, g2, w, 2), slots [w0, w0+w)
            return bass.AP(
                tensor=ap.tensor,
                offset=ap.offset + 2 * w0,
                ap=[ap.ap[0], [0, g2], [2, w], [1, 2]],
            )

        x_off = 0
        mi = 0
        pending = None
        # last matmul per bank under the tail rerouting below
        last_mi = {0: 0, 1: 0, 2: 0, 3: 0}
        for _mi in range(n_mm):
            _b = _mi % N_PSUM if _mi < n_mm - 32 else 2 + (_mi % 2)
            last_mi[_b] = _mi
        stop_mis = set(last_mi.values())
        early_stage_mis = {last_mi[0]: 0, last_mi[1]: 1}
        stage = singles.tile([16 * fold2 * 2, N_PSUM, NFINE * fold2 * 2], FP32)  # (a_t, d_t, G2) of the previous chunk
        def flush(pend):
            # cl + matmuls for a finished chunk; emitted one chunk late so
            # the in-order DVE never stalls waiting on ScalarE's d-slices.
            nonlocal mi
            a_t, d_t, G2, p2 = pend
            b_t = bpool.tile([P, G2, NFINE, 2], FP16, tag="b_t")
            if p2:
                nc.gpsimd.tensor_scalar(
                    b_t[:, :p2], d_t[:, :p2], 1.0, -1.0, ALU.min, ALU.max
                )
            nc.vector.tensor_scalar(
                b_t[:, p2:], d_t[:, p2:], 1.0, -1.0, ALU.min, ALU.max
            )
            for gb in range(0, G2, fold2):
                # Last 32 groups go to banks 2/3 only, so banks 0/1 retire
                # early and their PSUM->SBUF copies overlap the tail compute.
                if mi < n_mm - 32:
                    bank = mi % N_PSUM
                else:
                    bank = 2 + (mi % 2)
                nc.tensor.matmul(
                    psums[bank][:],
                    a_t[:, gb : gb + fold2, :, :],
                    b_t[:, gb : gb + fold2, :, :],
                    start=(mi < N_PSUM),
                    stop=(mi in stop_mis),
                )
                if mi in early_stage_mis:
                    b = early_stage_mis[mi]
                    nc.scalar.activation(stage[:, b, :], psums[b][:], ACT.Copy)
                mi += 1

        for c, (Gc, k_act) in enumerate(CHUNKS):
            G2 = Gc // 2
            p2 = 0  # GpSimd clamp disabled: its SBUF traffic stalls DVE
            xc = pool.tile([P, Gc], FP32, tag="xc")
            nc.sync.dma_start(xc[:], x_dram[:, x_off : x_off + Gc])
            x_off += Gc
            if not iota_loaded[0]:
                # after the first pixel DMA so chunk 0 hits ScalarE sooner
                nc.sync.dma_start(iota_a[:], iota_a_dram[:])
                nc.sync.dma_start(iota_b[:], iota_b_dram[:])
                iota_loaded[0] = True

            tb = spool.tile([P, Gc], FP16, tag="tb")
            if k_act:
                nc.scalar.activation(tb[:], xc[:], ACT.Copy, scale=255.0, bias=-8.0)
            else:
                nc.vector.tensor_scalar(
                    tb[:], xc[:], 255.0, 8.0, ALU.mult, ALU.subtract
                )
            h16 = spool.tile([P, Gc], FP16, tag="h16")
            nc.vector.tensor_scalar(
                h16[:], tb[:], MAGIC16, MAGIC16, ALU.add, ALU.subtract
            )
            u = spool.tile([P, Gc], FP16, tag="u")
            nc.vector.tensor_tensor(u[:], tb[:], h16[:], ALU.subtract)

            # d[l] = r - l = u - (l - 8): ScalarE owns the top k_act slots
            # (incl. the anchor) as Copy(bias=8-l); DVE owns the rest.
            ndve = NFINE - k_act
            d_t = pool.tile([P, G2, NFINE, 2], FP16, tag="d_t")
            for l in range(ndve, NFINE):
                bias = -4088.0 if l == NFINE - 1 else float(8 - l)
                nc.scalar.activation(
                    d_t[:, :, l, :], src_pairs(u[:], 1, G2), ACT.Copy, bias=bias
                )

            a_t = pool.tile([P, G2, 16, 2], FP16, tag="a_t")
            nc.vector.tensor_tensor(
                a_t[:],
                src_pairs(h16[:], 16, G2),
                iota_bcast(iota_a[:], 16, G2),
                ALU.is_equal,
            )
            nc.vector.tensor_tensor(
                d_t[:, :, :ndve, :],
                src_pairs(u[:], ndve, G2),
                iota_bcast(iota_b[:], ndve, G2),
                ALU.subtract,
            )

            if pending is not None:
                flush(pending)
            pending = (a_t, d_t, G2, p2)
        flush(pending)
        assert mi == n_mm, (mi, n_mm)

        # banks 0/1 were staged early (see loop); finish 2/3 and ship.
        for i in (2, 3):
            nc.scalar.activation(stage[:, i, :], psums[i][:], ACT.Copy)
        nc.sync.dma_start(gacc_dram.rearrange("n r c -> r n c"), stage[:])

    orig = nc.to_json_bytes
    nc.to_json_bytes = lambda *a, **k: _split_multiwaits(orig(*a, **k))
    return nc


def _gacc_to_hist(gacc: np.ndarray) -> np.ndarray:
    """(N_PSUM, 16*FOLD, 18*FOLD) raw accumulators -> (256,) histogram.

    Device accumulated G*[h,l] = sum_i A[i,h] * clamp(r_i - l, -1, 1) for
    l = 0..16, and G*[h,17] = -N_h (anchor column). Per pixel
    cl = 1 - 2*cumsum(B) + B, so PS[l] = (N_h - G*[l]) - PS[l-1] gives the
    prefix sums of G = A^T B, recovered by differencing."""
    fold2 = FOLD // 2
    acc = gacc.astype(np.float64).sum(axis=0)
    gst = np.zeros((16, NFINE), np.float64)
    for pb in range(fold2):
        for par in range(2):
            gst += acc[
                pb * 32 + par : pb * 32 + 32 : 2,
                pb * 2 * NFINE + par : pb * 2 * NFINE + 2 * NFINE : 2,
            ]
    nh = -gst[:, 17]
    ps = np.zeros((16, 17), np.float64)
    prev = np.zeros(16)
    for l in range(17):
        ps[:, l] = (nh - gst[:, l]) - prev
        prev = ps[:, l]
    g = np.diff(np.concatenate([np.zeros((16, 1)), ps], axis=1), axis=1)
    hist = g[:, :16].copy()
    hist[1:, 0] += g[:-1, 16]
    return hist.reshape(256).astype(np.float32)


_NC_CACHE = []


def kernel(images_batch: np.ndarray, bin_centers: np.ndarray) -> np.ndarray:
    images = np.asarray(images_batch, dtype=np.float32)
    assert images.shape == (N_CORES, 3, 256, 256), images.shape
    # bin_centers is linspace(0,1,256) by construction; the kernel math
    # hardcodes those bins (t = 255*x vs integer bin index).

    if not _NC_CACHE:
        _NC_CACHE.append(_build_program())
    nc = _NC_CACHE[0]

    in_maps = [{"x": images[b].reshape(P, F).copy()} for b in range(N_CORES)]
    res = run_bass_kernel_spmd(nc, in_maps, core_ids=list(range(N_CORES)))
    return np.stack([_gacc_to_hist(res.results[b]["gacc"]) for b in range(N_CORES)])


if __name__ == "__main__":
    rng = np.random.default_rng(1)
    imgs = rng.random((8, 3, 256, 256), dtype=np.float32)
    bins = np.linspace(0.0, 1.0, 256, dtype=np.float32)
    out = kernel(images_batch=imgs, bin_centers=bins)
    t = imgs.reshape(8, -1).astype(np.float64) * 255.0
    j = np.arange(256)
    want = np.clip(1.0 - np.abs(t[:, :, None] - j[None, None, :]), 0, None).sum(1)
    rel = np.abs(out - want).max() / np.abs(want).max()
    print("self-test rel err:", rel)
    print("PASS" if rel < 2e-2 else "FAIL")
